# revision 1
# baseline (speedup 1.0000x reference)
"""GQA causal attention (B=2, S=2048, H=2048, 32 Q heads / 8 KV heads, hd=64)
as an 8-way tensor-parallel Trainium2 Bass kernel.

Sharding: heads. Each NeuronCore gets 4 Q heads + their KV head (Wq/Wk/Wv
column slices, Wo row slice), computes a partial output over the full batch,
and the host sums the 8 partials (the Wo all-reduce done host-side).

Per-core dataflow (everything d-major / transposed so no on-device transposes
of activations are needed; host passes hidden pre-transposed):
    Q_T  = (Wq_c * scale)^T @ hidden^T        [256, B*S]
    KK_T = [Wk_c|Wk_c]^T @ hidden^T           [128, B*S] (duplicated halves so
                                              odd heads run on PE rows 64-127)
    V_T  = Wv_c^T @ hidden^T --PE-transpose-> V_aug [B*S, 65] (ones column
                                              accumulates the softmax denom)
    S_T[k,q] = K_T(chunk)^T x Q_T             only causal (lower) k-chunks
    P_T  = exp(S_T + tri-mask on diagonal chunks)      (no max-subtraction:
                                              scores are O(+-10), exp is safe)
    ctx_aug = V_aug^T @ P_T                   [65, q]; row 64 = denominator
    ctx  = ctx_aug[:64] * recip(denom)        stacked [256, q]
    out_partial = ctx^T @ Wo_c                [B*S, 2048]

All matmuls run as float32r (full-rate 1-cycle/row PE mode for fp32 data,
~1.5e-4 relative error measured on HW).
"""

import sys

for _p in ("/root/.axon_site", "/root/.axon_site/_ro/trn_rl_repo",
           "/root/.axon_site/_ro/pypackages", "/opt/trn_rl_repo", "/opt/pypackages"):
    if _p not in sys.path:
        sys.path.append(_p)

from contextlib import ExitStack

import numpy as np

import concourse.bass as bass  # noqa: F401
import concourse.tile as tile
from concourse import bacc, mybir
from concourse.bass_utils import run_bass_kernel_spmd

F32 = mybir.dt.float32
F32R = mybir.dt.float32r
P = 128
KC = 128
N_CORES = 8
HD = 64
NEG = -1e9

TRACE = False            # test harness flips this for NTFF profiling
TRACE_CORES = None
LAST_RESULT = None       # BassKernelResults of the last run (for the harness)

_nc_cache = {}


def build_attn_core(B=2, S=2048, H=2048, NHL=4, mask_mode="causal", QT=512,
                    debug_dump=False):
    """Build + bass-compile the per-core program.

    DRAM inputs (per core):
      ht  [H, B*S] f32r   hidden transposed      wq [H, NHL*HD] f32r (pre-scaled)
      wkv [H, 2*HD] f32r  [Wk_c | Wv_c]          wo [NHL*HD, H] f32r
      tri [KC, KC] f32    transposed causal block mask (tri[k,q]=0 iff k<=q)
      maskt [B, S, S] f32 (only mask_mode=="full") additive mask transposed
    Output: out_p [B*S, H] f32.
    """
    NQ = B * S
    CL = NHL * HD
    assert H % P == 0 and S % QT == 0 and QT % KC == 0 and NQ % QT == 0
    NHC = H // P
    NCC = CL // P
    QPB = S // QT
    KPB = S // KC
    DPT = QT // KC
    assert NHL % 2 == 0

    nc = bacc.Bacc("TRN2", target_bir_lowering=False, debug=False)

    ht = nc.dram_tensor("ht", [H, NQ], F32R, kind="ExternalInput").ap()
    wq = nc.dram_tensor("wq", [H, CL], F32R, kind="ExternalInput").ap()
    wkv = nc.dram_tensor("wkv", [H, 2 * HD], F32R, kind="ExternalInput").ap()
    wo = nc.dram_tensor("wo", [CL, H], F32R, kind="ExternalInput").ap()
    tri = nc.dram_tensor("tri", [KC, KC], F32, kind="ExternalInput").ap()
    ones = nc.dram_tensor("ones", [P, NQ // KC], F32R, kind="ExternalInput").ap()
    if mask_mode == "full":
        maskt = nc.dram_tensor("maskt", [B, S, S], F32, kind="ExternalInput").ap()
    out_p = nc.dram_tensor("out_p", [NQ, H], F32, kind="ExternalOutput").ap()

    with tile.TileContext(nc) as tc, ExitStack() as ctx:
        # ---- persistent SBUF ----
        pers = ctx.enter_context(tc.tile_pool(name="pers", bufs=1))
        wq_sb = pers.tile([P, NHC, CL], F32R, tag="wq")
        nc.sync.dma_start(wq_sb[:], wq.rearrange("(o p) m -> p o m", p=P))
        wkv_sb = pers.tile([P, NHC, 2 * HD], F32R, tag="wkv")
        nc.sync.dma_start(wkv_sb[:], wkv.rearrange("(o p) m -> p o m", p=P))
        wo_sb = pers.tile([P, NCC, H], F32R, tag="wo")
        nc.sync.dma_start(wo_sb[:], wo.rearrange("(o p) m -> p o m", p=P))
        tri_sb = pers.tile([KC, KC], F32, tag="tri")
        nc.sync.dma_start(tri_sb[:], tri)

        # identity (fp32) for PE transposes: keep diagonal 1.0, fill 0 off it
        ident = pers.tile([P, P], F32, tag="ident")
        nc.gpsimd.memset(ident[:], 1.0)
        nc.gpsimd.affine_select(
            out=ident[:], in_=ident[:],
            compare_op=mybir.AluOpType.is_equal, fill=0.0,
            base=0, pattern=[[-1, P]], channel_multiplier=1,
        )

        qt_sb = [pers.tile([P, NQ], F32R, tag=f"qt{c}", name=f"qt{c}")
                 for c in range(NCC)]
        kt_sb = pers.tile([P, NQ], F32R, tag="kt")          # [K_T ; K_T]
        v_sb = pers.tile([P, NQ // KC, HD + 1], F32R, tag="v")
        ctx_sb = pers.tile([P, NCC, QT], F32R, tag="ctx")

        # denom ones column (DMA'd: gpsimd memset can't write f32r)
        nc.sync.dma_start(v_sb[:, :, HD], ones)

        # ---- pools ----
        hpool = ctx.enter_context(tc.tile_pool(name="hpool", bufs=4))
        vtmp_pool = ctx.enter_context(tc.tile_pool(name="vtmp", bufs=2))
        pt_pool = ctx.enter_context(tc.tile_pool(name="pt", bufs=4))
        npool = ctx.enter_context(tc.tile_pool(name="npool", bufs=4))
        opool = ctx.enter_context(tc.tile_pool(name="opool", bufs=3))
        if mask_mode == "full":
            mpool = ctx.enter_context(tc.tile_pool(name="mpool", bufs=4))

        psA = ctx.enter_context(tc.tile_pool(name="psA", bufs=2, space="PSUM"))
        psB = ctx.enter_context(tc.tile_pool(name="psB", bufs=1, space="PSUM"))
        psS = ctx.enter_context(tc.tile_pool(name="psS", bufs=2, space="PSUM"))
        psC = ctx.enter_context(tc.tile_pool(name="psC", bufs=1, space="PSUM"))
        psO = ctx.enter_context(tc.tile_pool(name="psO", bufs=2, space="PSUM"))

        if debug_dump:
            dbg_qt = nc.dram_tensor("dbg_qt", [NCC, P, NQ], F32, kind="ExternalOutput").ap()
            dbg_kt = nc.dram_tensor("dbg_kt", [P, NQ], F32, kind="ExternalOutput").ap()
            dbg_v = nc.dram_tensor("dbg_v", [P, NQ // KC, HD + 1], F32, kind="ExternalOutput").ap()

        # ================= Phase A: projections =================
        NQT = NQ // QT
        for qt in range(NQT):
            q0 = qt * QT
            pq = [psA.tile([P, QT], F32, tag="pq", name=f"pq{i}") for i in range(NCC)]
            pkv = psB.tile([P, QT], F32, tag="pkv")
            for hc in range(NHC):
                h_t = hpool.tile([P, QT], F32R, tag="h")
                nc.sync.dma_start(h_t[:], ht[hc * P:(hc + 1) * P, q0:q0 + QT])
                fl = dict(start=(hc == 0), stop=(hc == NHC - 1))
                for cc in range(NCC):
                    nc.tensor.matmul(pq[cc][:], wq_sb[:, hc, cc * P:(cc + 1) * P],
                                     h_t[:], **fl)
                nc.tensor.matmul(pkv[:], wkv_sb[:, hc, :], h_t[:], **fl)
            for cc in range(NCC):
                nc.vector.tensor_copy(qt_sb[cc][:, q0:q0 + QT], pq[cc][:])
            # K_T rows 0-63; duplicate to 64-127 via SBUF->SBUF DMA
            nc.vector.tensor_copy(kt_sb[:HD, q0:q0 + QT], pkv[:HD, :])
            nc.sync.dma_start(kt_sb[HD:2 * HD, q0:q0 + QT], kt_sb[:HD, q0:q0 + QT])
            # V_T chunk -> PE-transpose into v_sb (natural [k, d] layout)
            vtmp = vtmp_pool.tile([P, QT], F32, tag="vt")
            nc.vector.tensor_copy(vtmp[HD:2 * HD, :], pkv[HD:2 * HD, :])
            for s4 in range(DPT):
                tp = psS.tile([P, QT], F32, tag="ps_s", name="tp")
                nc.tensor.transpose(
                    tp[:, :HD],
                    vtmp[HD:2 * HD, s4 * KC:(s4 + 1) * KC],
                    ident[HD:2 * HD, HD:2 * HD],
                )
                nc.vector.tensor_copy(v_sb[:, qt * DPT + s4, :HD], tp[:, :HD])

        if debug_dump:
            for c in range(NCC):
                nc.sync.dma_start(dbg_qt[c], qt_sb[c][:].bitcast(F32))
            nc.sync.dma_start(dbg_kt[:], kt_sb[:].bitcast(F32))
            nc.sync.dma_start(dbg_v[:], v_sb[:].bitcast(F32))

        # ================= Phase B: attention + out proj =================
        for b in range(B):
            for qtb in range(QPB):
                q0b = qtb * QT
                q0 = b * S + q0b
                nkc = (qtb + 1) * DPT if mask_mode == "causal" else KPB
                for h in range(NHL):
                    hb = (h % 2) * HD
                    cc = h // 2
                    cps = psC.tile([HD + 1, QT], F32, tag="ctx_ps")
                    for kc in range(nkc):
                        kcg = b * KPB + kc
                        diag_off = kc * KC - q0b
                        sps = psS.tile([P, QT], F32, tag="ps_s", name="sps")
                        nc.tensor.matmul(
                            sps[:],
                            kt_sb[hb:hb + HD, kcg * KC:(kcg + 1) * KC],
                            qt_sb[cc][hb:hb + HD, q0:q0 + QT],
                            start=True, stop=True,
                        )
                        pt = pt_pool.tile([P, QT], F32R, tag="pt")
                        if mask_mode == "full":
                            mt = mpool.tile([KC, QT], F32, tag="mt")
                            nc.sync.dma_start(
                                mt[:], maskt[b, kc * KC:(kc + 1) * KC, q0b:q0b + QT])
                            nc.vector.tensor_add(sps[:], sps[:], mt[:])
                            nc.scalar.activation(
                                pt[:], sps[:], mybir.ActivationFunctionType.Exp)
                        elif mask_mode == "causal" and diag_off >= 0:
                            # diagonal chunk: cols < diag_off fully masked,
                            # [diag_off, diag_off+KC) triangular, rest free
                            nc.vector.tensor_add(
                                sps[:, diag_off:diag_off + KC],
                                sps[:, diag_off:diag_off + KC],
                                tri_sb[:],
                            )
                            if diag_off > 0:
                                # cols left of the diagonal are fully masked:
                                # P = scores*0 = 0 (DVE can write f32r; memset can't)
                                nc.vector.tensor_scalar_mul(
                                    pt[:, :diag_off], sps[:, :diag_off], 0.0)
                            nc.scalar.activation(
                                pt[:, diag_off:], sps[:, diag_off:],
                                mybir.ActivationFunctionType.Exp,
                            )
                        else:
                            nc.scalar.activation(
                                pt[:], sps[:], mybir.ActivationFunctionType.Exp)
                        nc.tensor.matmul(
                            cps[:], v_sb[:, kcg, :], pt[:],
                            start=(kc == 0), stop=(kc == nkc - 1),
                        )
                    # normalize: ctx[:64] * recip(denom row). Denom is on PSUM
                    # partition 64; DVE is partition-locked, so recip stays on
                    # partition 64, a 2KB DMA moves it to partition 0, and
                    # gpsimd broadcasts it across partitions 0-63.
                    recip = npool.tile([P, QT], F32, tag="recip")
                    nc.vector.reciprocal(recip[HD:HD + 1, :], cps[HD:HD + 1, :])
                    nc.sync.dma_start(recip[0:1, :], recip[HD:HD + 1, :])
                    bcast = npool.tile([HD, QT], F32, tag="bcast")
                    nc.gpsimd.partition_broadcast(bcast[:], recip[0:1, :])
                    if h % 2 == 0:
                        nc.vector.tensor_mul(ctx_sb[:HD, cc, :], cps[:HD, :], bcast[:])
                    else:
                        ctmp = npool.tile([HD, QT], F32R, tag="ctmp")
                        nc.vector.tensor_mul(ctmp[:], cps[:HD, :], bcast[:])
                        nc.sync.dma_start(ctx_sb[HD:2 * HD, cc, :], ctmp[:])
                # ---- Wo ----
                ET = min(512, H)
                for qc in range(QT // P):
                    for et in range(H // ET):
                        po = psO.tile([P, ET], F32, tag="po")
                        for cc in range(NCC):
                            nc.tensor.matmul(
                                po[:],
                                ctx_sb[:, cc, qc * P:(qc + 1) * P],
                                wo_sb[:, cc, et * ET:(et + 1) * ET],
                                start=(cc == 0), stop=(cc == NCC - 1),
                            )
                        ob = opool.tile([P, ET], F32, tag="ob")
                        nc.vector.tensor_copy(ob[:], po[:])
                        nc.sync.dma_start(
                            out_p[q0 + qc * P:q0 + (qc + 1) * P, et * ET:(et + 1) * ET],
                            ob[:],
                        )

    nc.compile()
    return nc


def _detect_mask_mode(m, S):
    if not np.any(m):
        return "zeros"
    b0 = np.asarray(m[0, 0])
    qi = np.arange(S)
    tl = qi[None, :] <= qi[:, None]
    if (b0[tl] == 0.0).all() and (b0[~tl] <= -1e8).all() and (m == b0).all():
        return "causal"
    return "full"


def shard_inputs(hidden_states, attention_mask, Wq, Wk, Wv, Wo, mask_mode):
    B, S, H = hidden_states.shape
    NH = Wq.shape[1] // HD
    NKV = Wk.shape[1] // HD
    NHL = NH // N_CORES
    scale = np.float32(1.0 / np.sqrt(HD))

    ht = np.ascontiguousarray(
        hidden_states.reshape(B * S, H).T.astype(np.float32))
    if mask_mode == "causal":
        tri = np.ascontiguousarray(attention_mask[0, 0, :KC, :KC].T.astype(np.float32))
    else:
        tri = np.zeros((KC, KC), np.float32)
    if mask_mode == "full":
        maskt = np.ascontiguousarray(
            np.asarray(attention_mask)[:, 0].transpose(0, 2, 1).astype(np.float32))

    ones_np = np.ones((P, (B * S) // KC), np.float32)
    in_maps = []
    for c in range(N_CORES):
        wq_c = np.ascontiguousarray(
            Wq[:, c * NHL * HD:(c + 1) * NHL * HD].astype(np.float32) * scale)
        kv0 = c * (NKV // N_CORES) * HD
        wkv_c = np.ascontiguousarray(np.concatenate(
            [Wk[:, kv0:kv0 + HD], Wv[:, kv0:kv0 + HD]], axis=1).astype(np.float32))
        wo_c = np.ascontiguousarray(
            Wo[c * NHL * HD:(c + 1) * NHL * HD, :].astype(np.float32))
        im = {"ht": ht, "wq": wq_c, "wkv": wkv_c, "wo": wo_c, "tri": tri,
              "ones": ones_np}
        if mask_mode == "full":
            im["maskt"] = maskt
        in_maps.append(im)
    return in_maps, NHL


def kernel(hidden_states, attention_mask, Wq, Wk, Wv, Wo):
    global LAST_RESULT
    hidden_states = np.asarray(hidden_states, dtype=np.float32)
    attention_mask = np.asarray(attention_mask, dtype=np.float32)
    Wq, Wk, Wv, Wo = (np.asarray(w, dtype=np.float32) for w in (Wq, Wk, Wv, Wo))
    B, S, H = hidden_states.shape

    mask_mode = _detect_mask_mode(attention_mask, S)
    in_maps, NHL = shard_inputs(hidden_states, attention_mask, Wq, Wk, Wv, Wo,
                                mask_mode)

    key = (B, S, H, NHL, mask_mode)
    if key not in _nc_cache:
        _nc_cache[key] = build_attn_core(B=B, S=S, H=H, NHL=NHL,
                                         mask_mode=mask_mode)
    nc = _nc_cache[key]

    res = run_bass_kernel_spmd(nc, in_maps, core_ids=list(range(N_CORES)),
                               trace=TRACE, trace_cores=TRACE_CORES)
    LAST_RESULT = res

    out = res.results[0]["out_p"].astype(np.float32).copy()
    for c in range(1, N_CORES):
        out += res.results[c]["out_p"]
    return out.reshape(B, S, H)



# revision 17
# speedup vs baseline: 1.2281x; 1.2281x over previous
"""GQA causal attention (B=2, S=2048, H=2048, 32 Q heads / 8 KV heads, hd=64)
as an 8-way tensor-parallel Trainium2 Bass kernel.

Sharding: heads. Each NeuronCore gets 4 Q heads + their KV head (Wq/Wk/Wv
column slices, Wo row slice), computes a partial output over the full batch,
and the host sums the 8 partials (the Wo all-reduce done host-side).

v2: bf16 datapath end-to-end (halves DMA + DVE traffic, 1 cyc/row matmuls),
4 heads interleaved per k-chunk so the PE never waits on a single exp chain,
per-head PSUM ctx banks, batched reciprocal_approx_fast + PE-broadcast for
the softmax denominators, partial-width score/exp/AV on diagonal chunks.

Per-core dataflow (d-major / transposed; host passes hidden pre-transposed):
    Q_T  = (Wq_c * scale)^T @ hidden^T        [256, B*S]   (bf16)
    KK_T = [Wk_c|Wk_c]^T @ hidden^T           [128, B*S]   (dup halves so
                                              odd heads use PE rows 64-127)
    V_T  = Wv_c^T @ hidden^T --PE-transpose-> V_aug [B*S, 65] (ones column
                                              accumulates the softmax denom)
    S_T[k,q] = K_T(chunk)^T x Q_T             only causal (lower) k-chunks
    P_T  = exp(S_T + tri-mask on diagonal chunks)      (no max-subtraction:
                                              scores are O(+-10), exp is safe)
    ctx_aug = V_aug^T @ P_T                   [65, q]; row 64 = denominator
    ctx  = ctx_aug[:64] * bcast(recip(denom)) stacked [256, q]
    out_partial = ctx^T @ Wo_c                [B*S, 2048]  (bf16 out)
"""

import sys

for _p in ("/root/.axon_site", "/root/.axon_site/_ro/trn_rl_repo",
           "/root/.axon_site/_ro/pypackages", "/opt/trn_rl_repo", "/opt/pypackages"):
    if _p not in sys.path:
        sys.path.append(_p)

from contextlib import ExitStack

import numpy as np
import ml_dtypes

import concourse.bass as bass  # noqa: F401
import concourse.tile as tile
from concourse import bacc, mybir
from concourse.bass_utils import run_bass_kernel_spmd

F32 = mybir.dt.float32
F32R = mybir.dt.float32r
BF16 = mybir.dt.bfloat16
BF = ml_dtypes.bfloat16
P = 128
KC = 128
N_CORES = 8
HD = 64
NEG = -1e9

TRACE = False            # test harness flips this for NTFF profiling
TRACE_CORES = None
LAST_RESULT = None       # BassKernelResults of the last run (for the harness)

_nc_cache = {}


def build_attn_core(B=2, S=2048, H=2048, NHL=4, mask_mode="causal", QT=512,
                    debug_dump=False):
    """Build + bass-compile the per-core program.

    DRAM inputs (per core):
      ht  [H, B*S] bf16   hidden transposed      wq [H, NHL*HD] bf16 (pre-scaled)
      wkv [H, 2*HD] bf16  [Wk_c | Wv_c]          wo [NHL*HD, H] bf16
      tri [KC, KC] f32    transposed causal block mask (tri[k,q]=0 iff k<=q)
      onesv [P, B*S/KC] bf16   ones column for the V_aug denominator trick
      identv [HD, HD] f32      identity for the V PE-transposes
      maskt [B, S, S] f32 (only mask_mode=="full") additive mask transposed
    Output: out_p [B*S, H] bf16 (host upcasts + sums the 8 partials).
    """
    NQ = B * S
    CL = NHL * HD
    assert H % P == 0 and S % QT == 0 and QT % KC == 0 and NQ % QT == 0
    NHC = H // P
    NCC = CL // P
    QPB = S // QT
    KPB = S // KC
    DPT = QT // KC
    NKT = NQ // KC
    assert NHL % 2 == 0

    nc = bacc.Bacc("TRN2", target_bir_lowering=False, debug=False)

    ht = nc.dram_tensor("ht", [H, NQ], BF16, kind="ExternalInput").ap()
    wq = nc.dram_tensor("wq", [H, CL], BF16, kind="ExternalInput").ap()
    wkv = nc.dram_tensor("wkv", [H, 2 * HD], BF16, kind="ExternalInput").ap()
    wo = nc.dram_tensor("wo", [CL, H], BF16, kind="ExternalInput").ap()
    tri = nc.dram_tensor("tri", [KC, KC], F32, kind="ExternalInput").ap()
    onesv = nc.dram_tensor("onesv", [P, NKT], BF16, kind="ExternalInput").ap()
    identv = nc.dram_tensor("identv", [HD, HD], F32, kind="ExternalInput").ap()
    if mask_mode == "full":
        maskt = nc.dram_tensor("maskt", [B, S, S], F32, kind="ExternalInput").ap()
    out_p = nc.dram_tensor("out_p", [NQ, H], BF16, kind="ExternalOutput").ap()

    with tile.TileContext(nc) as tc, ExitStack() as ctx:
        # ---- persistent SBUF ----
        pers = ctx.enter_context(tc.tile_pool(name="pers", bufs=1))
        wq_sb = pers.tile([P, NHC, CL], BF16, tag="wq")
        nc.sync.dma_start(wq_sb[:], wq.rearrange("(o p) m -> p o m", p=P))
        wkv_sb = pers.tile([P, NHC, 2 * HD], BF16, tag="wkv")
        nc.sync.dma_start(wkv_sb[:], wkv.rearrange("(o p) m -> p o m", p=P))
        wo_sb = pers.tile([P, NCC, H], BF16, tag="wo")
        nc.sync.dma_start(wo_sb[:], wo.rearrange("(o p) m -> p o m", p=P))
        tri_sb = pers.tile([KC, KC], F32, tag="tri")
        nc.sync.dma_start(tri_sb[:], tri)
        ident_sb = pers.tile([P, HD], F32, tag="ident")
        nc.sync.dma_start(ident_sb[HD:2 * HD, :], identv)

        qt_sb = [pers.tile([P, NQ], BF16, tag=f"qt{c}", name=f"qt{c}")
                 for c in range(NCC)]
        kt_sb = pers.tile([P, NQ], BF16, tag="kt")          # [K_T ; K_T]
        v_sb = pers.tile([P, NKT, HD + 1], BF16, tag="v")
        ctx_sb = pers.tile([P, NCC, QT], BF16, tag="ctx")
        # softmax denominator reciprocals: computed on partition 64 (DVE is
        # partition-locked to the PSUM denom row), DMA'd to partition 0, then
        # gpsimd-broadcast across partitions 0-63
        rt_sb = pers.tile([P, NHL, QT], F32, tag="rt")
        lt_sb = pers.tile([P, QT], F32, tag="lt")

        # denom ones column of V_aug
        nc.sync.dma_start(v_sb[:, :, HD], onesv)

        # ================= Phase A: projections =================
        with tc.tile_pool(name="hpool", bufs=4) as hpool, \
             tc.tile_pool(name="vtp", bufs=2) as vtp, \
             tc.tile_pool(name="psA", bufs=4, space="PSUM") as psA, \
             tc.tile_pool(name="psB", bufs=2, space="PSUM") as psB, \
             tc.tile_pool(name="psT", bufs=2, space="PSUM") as psT:
            NQT = NQ // QT
            for g in range(NQT):
                q0 = g * QT
                pq = [psA.tile([P, QT], F32, tag="pq", name=f"pq{i}")
                      for i in range(NCC)]
                pkv = psB.tile([P, QT], F32, tag="pkv")
                for hc in range(NHC):
                    h_t = hpool.tile([P, QT], BF16, tag="h")
                    nc.sync.dma_start(h_t[:], ht[hc * P:(hc + 1) * P, q0:q0 + QT])
                    fl = dict(start=(hc == 0), stop=(hc == NHC - 1))
                    for cc in range(NCC):
                        nc.tensor.matmul(pq[cc][:], wq_sb[:, hc, cc * P:(cc + 1) * P],
                                         h_t[:], **fl)
                    nc.tensor.matmul(pkv[:], wkv_sb[:, hc, :], h_t[:], **fl)
                for cc in range(NCC):
                    nc.vector.tensor_copy(qt_sb[cc][:, q0:q0 + QT], pq[cc][:])
                # K_T rows 0-63; duplicate to 64-127 via SBUF->SBUF DMA
                nc.vector.tensor_copy(kt_sb[:HD, q0:q0 + QT], pkv[:HD, :])
                nc.sync.dma_start(kt_sb[HD:2 * HD, q0:q0 + QT], kt_sb[:HD, q0:q0 + QT])
                # V_T chunk -> PE-transpose into v_sb (natural [k, d] layout)
                vtmp = vtp.tile([P, QT], F32, tag="vt")
                nc.scalar.copy(vtmp[HD:2 * HD, :], pkv[HD:2 * HD, :])
                for s4 in range(DPT):
                    tp = psT.tile([P, HD], F32, tag="tp")
                    nc.tensor.transpose(
                        tp[:, :HD],
                        vtmp[HD:2 * HD, s4 * KC:(s4 + 1) * KC],
                        ident_sb[HD:2 * HD, :HD],
                    )
                    nc.vector.tensor_copy(v_sb[:, g * DPT + s4, :HD], tp[:, :HD])

        if debug_dump:
            dbg_qt = nc.dram_tensor("dbg_qt", [NCC, P, NQ], BF16,
                                    kind="ExternalOutput").ap()
            dbg_kt = nc.dram_tensor("dbg_kt", [P, NQ], BF16,
                                    kind="ExternalOutput").ap()
            dbg_v = nc.dram_tensor("dbg_v", [P, NKT, HD + 1], BF16,
                                   kind="ExternalOutput").ap()
            dbg_ctx = nc.dram_tensor("dbg_ctx", [P, NCC, QT], BF16,
                                     kind="ExternalOutput").ap()
            dbg_rt = nc.dram_tensor("dbg_rt", [P, NHL, QT], F32,
                                    kind="ExternalOutput").ap()
            for c in range(NCC):
                nc.sync.dma_start(dbg_qt[c], qt_sb[c][:])
            nc.sync.dma_start(dbg_kt[:], kt_sb[:])
            nc.sync.dma_start(dbg_v[:], v_sb[:])

        # ================= Phase B: attention + out proj =================
        with ExitStack() as bctx:
            ptp = bctx.enter_context(tc.tile_pool(name="ptp", bufs=4))
            ctp = bctx.enter_context(tc.tile_pool(name="ctp", bufs=2))
            opool = bctx.enter_context(tc.tile_pool(name="opool", bufs=4))
            psS = bctx.enter_context(tc.tile_pool(name="psS", bufs=2, space="PSUM"))
            psC = bctx.enter_context(tc.tile_pool(name="psC", bufs=4, space="PSUM"))
            psW = bctx.enter_context(tc.tile_pool(name="psW", bufs=2, space="PSUM"))
            if mask_mode == "full":
                mpool = bctx.enter_context(tc.tile_pool(name="mpool", bufs=4))

            for b in range(B):
                for qtb in range(QPB):
                    q0b = qtb * QT
                    q0 = b * S + q0b
                    nkc = (qtb + 1) * DPT if mask_mode == "causal" else KPB
                    cps = [psC.tile([HD + 1, QT], F32, tag="cps", name=f"cps{h}")
                           for h in range(NHL)]
                    for kc in range(nkc):
                        kcg = b * KPB + kc
                        diag_off = kc * KC - q0b
                        is_diag = mask_mode == "causal" and diag_off >= 0
                        lo = diag_off if is_diag else 0
                        pts = []
                        for h in range(NHL):
                            hb = (h % 2) * HD
                            cc = h // 2
                            sps = psS.tile([P, QT], F32, tag="sps")
                            nc.tensor.matmul(
                                sps[:, lo:],
                                kt_sb[hb:hb + HD, kcg * KC:(kcg + 1) * KC],
                                qt_sb[cc][hb:hb + HD, q0 + lo:q0 + QT],
                                start=True, stop=True,
                            )
                            pt = ptp.tile([P, QT], BF16, tag="pt")
                            if mask_mode == "full":
                                mt = mpool.tile([KC, QT], F32, tag="mt")
                                nc.sync.dma_start(
                                    mt[:], maskt[b, kc * KC:(kc + 1) * KC, q0b:q0b + QT])
                                nc.vector.tensor_add(sps[:], sps[:], mt[:])
                                nc.scalar.activation(
                                    pt[:], sps[:], mybir.ActivationFunctionType.Exp)
                            elif is_diag:
                                nc.vector.tensor_add(
                                    sps[:, diag_off:diag_off + KC],
                                    sps[:, diag_off:diag_off + KC],
                                    tri_sb[:],
                                )
                                nc.scalar.activation(
                                    pt[:, lo:], sps[:, lo:],
                                    mybir.ActivationFunctionType.Exp,
                                )
                            else:
                                nc.scalar.activation(
                                    pt[:], sps[:], mybir.ActivationFunctionType.Exp)
                            pts.append((pt, lo))
                        for h in range(NHL):
                            pt, lo = pts[h]
                            nc.tensor.matmul(
                                cps[h][:, lo:], v_sb[:, kcg, :], pt[:, lo:],
                                start=(kc == 0), stop=(kc == nkc - 1),
                                skip_group_check=True,
                            )
                    # ---- normalize: 1/x = exp(-ln x) on ScalarE (both funcs
                    # live in the natural_log_exp_and_others table set, so no
                    # ACT table switching), then gpsimd broadcast ----
                    for h in range(NHL):
                        nc.scalar.activation(
                            lt_sb[HD:HD + 1, :], cps[h][HD:HD + 1, :],
                            mybir.ActivationFunctionType.Ln)
                        nc.scalar.activation(
                            rt_sb[HD:HD + 1, h, :], lt_sb[HD:HD + 1, :],
                            mybir.ActivationFunctionType.Exp, scale=-1.0)
                        nc.sync.dma_start(rt_sb[0:1, h, :], rt_sb[HD:HD + 1, h, :])
                    for h in range(NHL):
                        cc = h // 2
                        bcs = ctp.tile([HD, QT], F32, tag="bcs")
                        nc.gpsimd.partition_broadcast(bcs[:], rt_sb[0:1, h, :])
                        if h % 2 == 0:
                            nc.vector.tensor_mul(ctx_sb[:HD, cc, :], cps[h][:HD, :],
                                                 bcs[:])
                        else:
                            ctmp = ctp.tile([HD, QT], BF16, tag="ctmp")
                            nc.vector.tensor_mul(ctmp[:], cps[h][:HD, :], bcs[:])
                            nc.sync.dma_start(ctx_sb[HD:2 * HD, cc, :], ctmp[:])
                    if debug_dump and b == 0 and qtb == 0:
                        nc.sync.dma_start(dbg_ctx[:], ctx_sb[:])
                        nc.sync.dma_start(dbg_rt[:], rt_sb[:])
                    # ---- Wo ----
                    ET = min(512, H)
                    for qc in range(QT // P):
                        for et in range(H // ET):
                            po = psW.tile([P, ET], F32, tag="po")
                            for cc in range(NCC):
                                nc.tensor.matmul(
                                    po[:],
                                    ctx_sb[:, cc, qc * P:(qc + 1) * P],
                                    wo_sb[:, cc, et * ET:(et + 1) * ET],
                                    start=(cc == 0), stop=(cc == NCC - 1),
                                )
                            ob = opool.tile([P, ET], BF16, tag="ob")
                            eng = nc.vector if et % 2 == 0 else nc.scalar
                            if eng is nc.vector:
                                eng.tensor_copy(ob[:], po[:])
                            else:
                                eng.copy(ob[:], po[:])
                            nc.sync.dma_start(
                                out_p[q0 + qc * P:q0 + (qc + 1) * P,
                                      et * ET:(et + 1) * ET],
                                ob[:],
                            )

    nc.compile()
    return nc


def _detect_mask_mode(m, S):
    if not np.any(m):
        return "zeros"
    b0 = np.asarray(m[0, 0])
    qi = np.arange(S)
    tl = qi[None, :] <= qi[:, None]
    if (b0[tl] == 0.0).all() and (b0[~tl] <= -1e8).all() and (m == b0).all():
        return "causal"
    return "full"


def shard_inputs(hidden_states, attention_mask, Wq, Wk, Wv, Wo, mask_mode):
    B, S, H = hidden_states.shape
    NH = Wq.shape[1] // HD
    NKV = Wk.shape[1] // HD
    NHL = NH // N_CORES
    scale = np.float32(1.0 / np.sqrt(HD))

    ht = np.ascontiguousarray(
        hidden_states.reshape(B * S, H).T.astype(np.float32)).astype(BF)
    if mask_mode == "causal":
        tri = np.ascontiguousarray(attention_mask[0, 0, :KC, :KC].T.astype(np.float32))
    else:
        tri = np.zeros((KC, KC), np.float32)
    if mask_mode == "full":
        maskt = np.ascontiguousarray(
            np.asarray(attention_mask)[:, 0].transpose(0, 2, 1).astype(np.float32))

    onesv_np = np.ones((P, (B * S) // KC), BF)
    identv_np = np.eye(HD, dtype=np.float32)
    in_maps = []
    for c in range(N_CORES):
        wq_c = np.ascontiguousarray(
            Wq[:, c * NHL * HD:(c + 1) * NHL * HD].astype(np.float32)
            * scale).astype(BF)
        kv0 = c * (NKV // N_CORES) * HD
        wkv_c = np.ascontiguousarray(np.concatenate(
            [Wk[:, kv0:kv0 + HD], Wv[:, kv0:kv0 + HD]],
            axis=1).astype(np.float32)).astype(BF)
        wo_c = np.ascontiguousarray(
            Wo[c * NHL * HD:(c + 1) * NHL * HD, :].astype(np.float32)).astype(BF)
        im = {"ht": ht, "wq": wq_c, "wkv": wkv_c, "wo": wo_c, "tri": tri,
              "onesv": onesv_np, "identv": identv_np}
        if mask_mode == "full":
            im["maskt"] = maskt
        in_maps.append(im)
    return in_maps, NHL


def kernel(hidden_states, attention_mask, Wq, Wk, Wv, Wo):
    global LAST_RESULT
    hidden_states = np.asarray(hidden_states, dtype=np.float32)
    attention_mask = np.asarray(attention_mask, dtype=np.float32)
    Wq, Wk, Wv, Wo = (np.asarray(w, dtype=np.float32) for w in (Wq, Wk, Wv, Wo))
    B, S, H = hidden_states.shape

    mask_mode = _detect_mask_mode(attention_mask, S)
    in_maps, NHL = shard_inputs(hidden_states, attention_mask, Wq, Wk, Wv, Wo,
                                mask_mode)

    key = (B, S, H, NHL, mask_mode)
    if key not in _nc_cache:
        _nc_cache[key] = build_attn_core(B=B, S=S, H=H, NHL=NHL,
                                         mask_mode=mask_mode)
    nc = _nc_cache[key]

    res = run_bass_kernel_spmd(nc, in_maps, core_ids=list(range(N_CORES)),
                               trace=TRACE, trace_cores=TRACE_CORES)
    LAST_RESULT = res

    out = res.results[0]["out_p"].astype(np.float32)
    for c in range(1, N_CORES):
        out = out + res.results[c]["out_p"].astype(np.float32)
    return out.reshape(B, S, H)
